# revision 41
# baseline (speedup 1.0000x reference)
"""Multi-head attention (B=2, N=2048, dim=1024, heads=16, dim_head=64) on
8 TRN2 NeuronCores.

Sharding: data-parallel over batch (2) x tensor-parallel over heads (4 per
core).  Core c handles batch b = c//4 and heads [4g, 4g+4), g = c%4.  Each
core computes its 4 heads' attention plus the partial out-projection; the
host sums the 4 partials per batch and adds the bias.

Pipeline design (vs. the previous version):
  * Streaming prologue: xt arrives chunk-by-chunk over 3 DMA launcher
    engines (sync/gpsimd/scalar); Kt/V tiles are produced just-in-time as
    fillers inside the attention loop, so the first exp fires at ~10us
    instead of ~40us.
  * exp batching: St for an mt-PAIR lands in one [128, 1024] 2-bank PSUM
    tile; one ACTIVATE covers both tiles -> ScalarE overhead per element
    drops ~20%.
  * AV lags St by 2 mt-pairs so filler DMA waits never starve ScalarE.
  * Output staged in fp16 (half the writeback traffic); host accumulates
    partials in fp32.

Per-iteration steady state (one mt-pair, one head pair, one 512-query
chunk): PE: 4 AV MMs (lag 2) + 4 St MMs + fillers; ScalarE: 2 x
exp([128,1024]); DVE/GpSimd: evacuations + normalize.  PSUM: st 2x
[128,1024] (4 banks) + ot 2x[65,512] (2) + qk 2x[128,512] (2) = 8 banks.
"""
import numpy as np

import concourse.bass as bass
import concourse.mybir as mybir
import concourse.tile as tile
from concourse import bacc
from concourse.bass_utils import run_bass_kernel_spmd

# Problem constants (hardcoded per contract).
B = 2
N = 2048
DIM = 1024
HEADS = 16
DH = 64
INNER = HEADS * DH
SCALE = DH ** -0.5

N_CORES = 8
HEADS_PER_CORE = 4
PAIRS = 2          # head pairs per core
NT = N // 128      # 16 key tiles
DT = DIM // 128    # 8 contraction tiles
CH = N // 512      # 4 query chunks
MTP = NT // 2      # 8 key-tile pairs per chunk
F32 = mybir.dt.float32
F16 = mybir.dt.float16
BF16 = mybir.dt.bfloat16

# wqkv column layout (host-packed): [k_p0 | q_p0 | v | k_p1 | q_p1]
KCOL = {0: 0, 1: 512}
QCOL = {0: 128, 1: 640}
VCOL = 256

_CACHED_NC = None


def _emit_kernel(tc, xt_d, wqkv_d, wo_d, eye_d, out_d):
    nc = tc.nc

    from contextlib import ExitStack

    ctx = ExitStack()
    per = ctx.enter_context(tc.tile_pool(name="persist", bufs=1))
    psum = ctx.enter_context(tc.tile_pool(name="psum", bufs=1, space="PSUM"))
    work = ctx.enter_context(tc.tile_pool(name="work", bufs=1))

    # Persistent SBUF tensors.
    xt_sb = per.tile([128, DT, N], BF16, tag="xt")
    wqkv_sb = per.tile([128, DT, 768], BF16, tag="wqkv")
    wo_sb = per.tile([128, PAIRS, DIM], BF16, tag="wo")
    qt_sb = per.tile([128, PAIRS, N], BF16, tag="qt")
    kt_sb = per.tile([128, PAIRS, N], BF16, tag="kt")
    v_sb = per.tile([128, NT, HEADS_PER_CORE, DH + 1], BF16, tag="v")
    o_sb = per.tile([128, PAIRS, N], BF16, tag="o")

    # ---- Input DMAs, split across launcher engines so the first chunk of
    # xt plus the pair-0 QK weights land in ~6us.  Each launch is one
    # descriptor chain on one HW queue; sync issues the early-needed data.
    # xt_d and wqkv_d are host-packed in their SBUF tile layouts
    # ([128, DT, N] / [128, DT, 768], flattened) so every DMA below is a
    # 1:1 strided slice copy.
    # sync+scalar share one 8-queue HWDGE pool; gpsimd has its own.  Wave 1
    # fills all 16 queues with exactly the early-needed 2.9MB (kq-p0
    # weights, xt c0, v weights, xt c1); everything else rides behind in
    # the same queues' FIFOs.  Per-queue bandwidth is modest (~40GB/s), so
    # xt c0 is spread over 8 queues (4 sync + 4 scalar).
    xt_src = xt_d.rearrange("p (d c) -> p d c", c=N)
    wqkv_src = wqkv_d.rearrange("p (d c) -> p d c", c=768)
    for t in range(2):       # k_p0 | q_p0 weight block (Kt consumes dt-major)
        nc.sync.dma_start(
            wqkv_sb[:, 4 * t:4 * t + 4, 0:256], wqkv_src[:, 4 * t:4 * t + 4, 0:256]
        )
    eye_sb = per.tile([128, 128], BF16, tag="eye")
    nc.scalar.dma_start(eye_sb[:], eye_d[:, :])   # identity for PE transposes
    for t in range(4):       # xt chunk 0, dt 0-3 (sync) + dt 4-7 (scalar)
        nc.sync.dma_start(xt_sb[:, t, 0:512], xt_src[:, t, 0:512])
        nc.scalar.dma_start(xt_sb[:, 4 + t, 0:512], xt_src[:, 4 + t, 0:512])
    for t in range(2):       # v weights (gpsimd queue pool)
        nc.gpsimd.dma_start(
            wqkv_sb[:, 4 * t:4 * t + 4, 256:512],
            wqkv_src[:, 4 * t:4 * t + 4, 256:512],
        )
    for t in range(4):       # xt chunk 1
        nc.gpsimd.dma_start(
            xt_sb[:, 2 * t:2 * t + 2, 512:1024],
            xt_src[:, 2 * t:2 * t + 2, 512:1024],
        )
    # -- second wave: rides behind the first in the queue FIFOs --
    for t in range(4):       # xt chunk 2
        nc.gpsimd.dma_start(
            xt_sb[:, 2 * t:2 * t + 2, 1024:1536],
            xt_src[:, 2 * t:2 * t + 2, 1024:1536],
        )
    for t in range(4):       # xt chunk 3
        nc.sync.dma_start(
            xt_sb[:, 2 * t:2 * t + 2, 1536:2048],
            xt_src[:, 2 * t:2 * t + 2, 1536:2048],
        )
    for t in range(2):       # k_p1 | q_p1 weights
        nc.gpsimd.dma_start(
            wqkv_sb[:, 4 * t:4 * t + 4, 512:768],
            wqkv_src[:, 4 * t:4 * t + 4, 512:768],
        )
    for p in range(PAIRS):   # out-projection weights
        nc.gpsimd.dma_start(wo_sb[:, p, :], wo_d[128 * p:128 * (p + 1), :])

    # Ones column of V' (gives the softmax denominator through the AV matmul).
    ones_sb = per.tile([128, NT * HEADS_PER_CORE], F32, tag="ones")
    nc.vector.memset(ones_sb[:], 1.0)
    nc.vector.tensor_copy(
        v_sb[:, :, :, DH:DH + 1],
        ones_sb[:].rearrange("p (a b c) -> p a b c", b=HEADS_PER_CORE, c=1),
    )
    # Touch Exp once so the ACT table DMA (~2.7us) happens during startup.
    warm = work.tile([1, 1], F32, tag="warm")
    nc.scalar.activation(
        warm[:], ones_sb[0:1, 0:1], mybir.ActivationFunctionType.Exp, scale=1.0
    )
    # Warm the HAM clock gate while input DMAs land.  bf16 dummies fed from
    # the just-DMA'd weight block: no DVE dependency (DVE is stuck on its
    # table load for the first ~4us), start as soon as the first weight DMA
    # lands (~1.5us), and roll straight into the real Kt matmuls.
    for i in range(12):
        dummy = psum.tile([64, 64], F32, tag="qk", bufs=2, name="dummy")
        nc.tensor.matmul(
            dummy[:], wqkv_sb[0:64, 0, 0:64], wqkv_sb[0:64, 0, 0:64],
            start=True, stop=True,
        )

    def emit_qk_chunk(which, p, c):
        """Qt or Kt for head pair p, n-chunk c: [128, 512] of W.T @ xT."""
        src = qt_sb if which == "q" else kt_sb
        col0 = (QCOL if which == "q" else KCOL)[p]
        ps = psum.tile([128, 512], F32, tag="qk", bufs=2)
        for dt in range(DT):
            nc.tensor.matmul(
                ps[:],
                wqkv_sb[:, dt, col0:col0 + 128],
                xt_sb[:, dt, 512 * c:512 * (c + 1)],
                start=(dt == 0),
                stop=(dt == DT - 1),
            )
        nc.vector.tensor_copy(src[:, p, 512 * c:512 * (c + 1)], ps[:])

    def emit_vt_block(vb, mc):
        """V for heads 2vb,2vb+1 over m-chunk mc, via W-stationary V^T
        matmuls (dense N=512 streams; ~2x fewer PE cycles than the
        x-stationary N=256 form) + PE transposes back to natural layout."""
        ps = psum.tile([128, 512], F32, tag="qk", bufs=2)
        for dt in range(DT):
            nc.tensor.matmul(
                ps[:],
                wqkv_sb[:, dt, VCOL + 128 * vb:VCOL + 128 * (vb + 1)],
                xt_sb[:, dt, 512 * mc:512 * (mc + 1)],
                start=(dt == 0),
                stop=(dt == DT - 1),
            )
        vt = work.tile([128, 512], BF16, tag="vt", bufs=4, name="vt")
        nc.vector.tensor_copy(vt[:], ps[:])
        for q in range(4):
            mt = 4 * mc + q
            tp = psum.tile([128, 128], BF16, tag="qk", bufs=2, name="tp")
            nc.tensor.transpose(tp[:], vt[:, 128 * q:128 * (q + 1)], eye_sb[:])
            nc.vector.tensor_copy(
                v_sb[:, mt, 2 * vb:2 * vb + 2, 0:DH],
                tp[:].rearrange("p (h d) -> p h d", h=2),
            )

    ev_tiles = {}

    def emit_proj_unit(nt, jc, evac=None):
        """out[128nt:+128, 512jc:+512] = sum_p o_sb[:,p,nt].T @ wo[:,p,jc]."""
        if nt not in ev_tiles:
            ev_tiles[nt] = work.tile([128, DIM], F16, tag="ev", bufs=3, name="ev")
        ev = ev_tiles[nt]
        ps = psum.tile([128, 512], F32, tag="qk", bufs=2)
        for p in range(PAIRS):
            nc.tensor.matmul(
                ps[:],
                o_sb[:, p, 128 * nt:128 * (nt + 1)],
                wo_sb[:, p, 512 * jc:512 * (jc + 1)],
                start=(p == 0),
                stop=(p == PAIRS - 1),
            )
        if evac == "scalar":
            nc.scalar.copy(ev[:, 512 * jc:512 * (jc + 1)], ps[:])
        else:
            nc.vector.tensor_copy(ev[:, 512 * jc:512 * (jc + 1)], ps[:])
        if jc == 1:
            nc.sync.dma_start(out_d[128 * nt:128 * (nt + 1), :], ev[:])
            del ev_tiles[nt]

    def run_filler(unit):
        kind = unit[0]
        if kind == "v":
            emit_vt_block(unit[1], unit[2])
        elif kind == "qk":
            emit_qk_chunk(unit[1], unit[2], unit[3])
        else:
            emit_proj_unit(unit[1], unit[2], evac=unit[3])

    def emit_normalize(p, c, ot):
        """o = Ot'[0:64] / Ot'[64], interleaved across heads."""
        den, recip, rbc = [None, None], [None, None], [None, None]
        for h in range(2):
            den[h] = work.tile([1, 512], F32, tag="den", bufs=4, name=f"den{h}")
            nc.vector.tensor_copy(den[h][:], ot[h][DH:DH + 1, :])
        for h in range(2):
            recip[h] = work.tile([1, 512], F32, tag="recip", bufs=4, name=f"rec{h}")
            nc.vector.reciprocal_approx_fast(recip[h][:], den[h][:])
        for h in range(2):
            rbc[h] = work.tile([64, 512], F32, tag="rbc", bufs=4, name=f"rbc{h}")
            nc.gpsimd.partition_broadcast(rbc[h][:], recip[h][:])
        for h in range(2):
            nc.vector.tensor_mul(
                o_sb[64 * h:64 * (h + 1), p, 512 * c:512 * (c + 1)],
                ot[h][0:DH, :],
                rbc[h][:],
            )

    # The St -> exp -> AV pipeline runs CONTINUOUSLY across chunks and
    # head pairs: AV for mt-pair k flushes while mt-pair k+2 is being
    # scored (2-iteration lag), including across chunk boundaries, so the
    # PE never drains waiting for the tail exps of a chunk.  A chunk's
    # normalize is emitted right after its last AV flush (early in the
    # next chunk).
    pend = []
    norm_q = []   # completed (p, c, ot) chunks awaiting normalize

    def flush_one():
        p, c, mtp, pts, ot = pend.pop(0)
        for h in range(2):
            for j in range(2):
                mt = 2 * mtp + j
                nc.tensor.matmul(
                    ot[h][:],
                    v_sb[:, mt, 2 * p + h, :],
                    pts[h][:, 512 * j:512 * (j + 1)],
                    start=(mt == 0),
                    stop=(mt == NT - 1),
                )
        if mtp == MTP - 1:
            norm_q.append((p, c, ot))

    def emit_att_chunk(p, c, fillers):
        ot = [
            psum.tile([DH + 1, 512], F32, tag="ot", bufs=2, name=f"ot{h}")
            for h in range(2)
        ]
        for mtp in range(MTP):
            if norm_q and mtp == 2:
                # Normalize the previous chunk a couple of iterations in,
                # so its DVE burst doesn't delay the boundary Qt evac.
                # Must precede this chunk's first AV flush (start=True),
                # which reuses the previous chunk's rotated ot slots.
                emit_normalize(*norm_q.pop(0))
            if len(pend) == 2:
                flush_one()
            st = [None, None]
            for h in range(2):
                st[h] = psum.tile([128, 1024], F32, tag="st", bufs=2, name=f"st{h}")
                for j in range(2):
                    mt = 2 * mtp + j
                    nc.tensor.matmul(
                        st[h][:, 512 * j:512 * (j + 1)],
                        kt_sb[64 * h:64 * (h + 1), p, 128 * mt:128 * (mt + 1)],
                        qt_sb[64 * h:64 * (h + 1), p, 512 * c:512 * (c + 1)],
                        start=True,
                        stop=True,
                        tile_position=(64 * h, 0),
                    )
            pts = [None, None]
            for h in range(2):
                pts[h] = work.tile([128, 1024], BF16, tag="pt", bufs=8, name=f"pt{h}")
                nc.scalar.activation(
                    pts[h][:], st[h][:], mybir.ActivationFunctionType.Exp,
                    scale=SCALE,
                )
            pend.append((p, c, mtp, pts, ot))
            for unit in fillers.get(mtp, ()):
                run_filler(unit)
        for unit in fillers.get(MTP, ()):
            run_filler(unit)

    # ---- Emission schedule ----
    # Prologue: just enough for the first St + the first AV flushes.
    # VT blocks (0,0)/(1,0) cover key tiles 0-3 for all four heads.
    emit_qk_chunk("k", 0, 0)
    emit_qk_chunk("q", 0, 0)
    emit_vt_block(0, 0)
    emit_vt_block(1, 0)

    # Pair 0.  VT m-chunk blocks and Kt chunks stream ~2 iterations ahead
    # of their AV/St consumers; xt chunk k lands at roughly 10+3k us.
    p0_fill = {
        0: {0: [("qk", "k", 0, 1)], 1: [("v", 0, 1)], 2: [("v", 1, 1)],
            3: [("qk", "k", 0, 2)], 4: [("v", 0, 2), ("v", 1, 2)],
            5: [("qk", "k", 0, 3)], 6: [("v", 0, 3), ("qk", "q", 0, 1)],
            7: [("v", 1, 3)]},
        1: {0: [("qk", "k", 1, 0)], 2: [("qk", "k", 1, 1)],
            4: [("qk", "k", 1, 2), ("qk", "q", 0, 2)], 6: [("qk", "k", 1, 3)]},
        2: {1: [("qk", "q", 1, 0)], 3: [("qk", "q", 0, 3)]},
        3: {1: [("qk", "q", 1, 1)], 5: [("qk", "q", 1, 2)]},
    }
    for c in range(CH):
        emit_att_chunk(0, c, p0_fill[c])

    # Pair 1.  Chunk c's projection units run as fillers inside chunk c+1.
    def proj_units(nts, evac=None):
        return [("proj", nt, jc, evac) for nt in nts for jc in range(2)]

    # Chunk c-1's o is normalized early in pair-1 chunk c (after the lagged
    # AV drain), so its projection units start at iteration 2.
    def spread_proj(nts):
        sched = {2: [], 3: [], 4: [], 5: [], 6: [], 7: []}
        for i, u in enumerate(proj_units(nts)):
            sched[2 + i * 6 // 8].append(u)
        return sched

    p1_fill = {
        0: {1: [("qk", "q", 1, 3)]},
        1: spread_proj(range(0, 4)),
        2: spread_proj(range(4, 8)),
        3: spread_proj(range(8, 12)),
    }
    for c in range(CH):
        emit_att_chunk(1, c, p1_fill[c])
    while pend:
        flush_one()
    while norm_q:
        emit_normalize(*norm_q.pop(0))
    # Tail drain: remaining projections with evacuations spread across
    # ScalarE / DVE (ScalarE is idle now).
    evacs = ["scalar", None, "scalar", None, "scalar", None, "scalar", None]
    for i, (nt, jc) in enumerate([(nt, jc) for nt in range(12, 16) for jc in range(2)]):
        emit_proj_unit(nt, jc, evac=evacs[i])

    ctx.close()


def _build():
    global _CACHED_NC
    if _CACHED_NC is not None:
        return _CACHED_NC
    nc = bacc.Bacc(
        "TRN2",
        target_bir_lowering=False,
        debug=False,
        enable_asserts=True,
        num_devices=N_CORES,
    )
    xt_d = nc.dram_tensor("xt", [128, DT * N], BF16, kind="ExternalInput").ap()
    wqkv_d = nc.dram_tensor("wqkv", [128, DT * 768], BF16, kind="ExternalInput").ap()
    wo_d = nc.dram_tensor("wo", [256, DIM], BF16, kind="ExternalInput").ap()
    eye_d = nc.dram_tensor("eye", [128, 128], BF16, kind="ExternalInput").ap()
    out_d = nc.dram_tensor("out", [N, DIM], F16, kind="ExternalOutput").ap()

    with tile.TileContext(nc) as tc:
        _emit_kernel(tc, xt_d, wqkv_d, wo_d, eye_d, out_d)
    nc.compile()
    _CACHED_NC = nc
    return nc


def _in_maps(x, w_qkv, w_out):
    import ml_dtypes

    bf = ml_dtypes.bfloat16
    maps = []
    for c in range(N_CORES):
        b, g = divmod(c, 4)
        cols = slice(256 * g, 256 * (g + 1))
        q = w_qkv[:, cols]
        k = w_qkv[:, INNER:][:, cols]
        v = w_qkv[:, 2 * INNER:][:, cols]
        # Column layout [k_p0 | q_p0 | v | k_p1 | q_p1] so the earliest-
        # needed weights are first in DMA order; then repacked into the
        # SBUF tile layout [128, DT, 768] so device DMAs are 1:1 slices.
        wqkv_c = np.concatenate(
            [k[:, 0:128], q[:, 0:128], v, k[:, 128:256], q[:, 128:256]],
            axis=1,
        )
        wqkv_c = np.ascontiguousarray(
            wqkv_c.reshape(DT, 128, 768).transpose(1, 0, 2).reshape(128, DT * 768)
            .astype(bf)
        )
        xt_c = np.ascontiguousarray(
            x[b].T.reshape(DT, 128, N).transpose(1, 0, 2).reshape(128, DT * N)
            .astype(bf)
        )
        maps.append(
            {
                "xt": xt_c,
                "wqkv": wqkv_c,
                "wo": np.ascontiguousarray(w_out[cols, :].astype(bf)),
                "eye": np.eye(128, dtype=bf),
            }
        )
    return maps


def _run(x, w_qkv, w_out, b_out, trace=False):
    nc = _build()
    res = run_bass_kernel_spmd(
        nc, _in_maps(x, w_qkv, w_out), list(range(N_CORES)), trace=trace
    )
    partials = np.stack(
        [res.results[c]["out"].astype(np.float32) for c in range(N_CORES)]
    )
    out = np.empty((B, N, DIM), dtype=np.float32)
    for b in range(B):
        out[b] = partials[4 * b:4 * b + 4].sum(axis=0) + b_out
    return out, res


def kernel(x, w_qkv, w_out, b_out):
    out, _ = _run(
        np.asarray(x, dtype=np.float32),
        np.asarray(w_qkv, dtype=np.float32),
        np.asarray(w_out, dtype=np.float32),
        np.asarray(b_out, dtype=np.float32),
    )
    return out
